# revision 17
# baseline (speedup 1.0000x reference)
"""Trainium2 Bass kernel for nn_Attention_86268713108190.

7 independent attention "bands" over batch 8, n=512, d=512, 8 heads,
shared Wqkv/Wout. Sharding: data-parallel over batch -- core c handles
batch index c (7 band-samples of [512, 512] each).

Design (HW-measured 274us/core vs the 551us f32r baseline, rel err 4.6e-3
vs the 2e-2 gate; all decomposition numbers below are For_i
repeat-differenced on the axon-tunneled TRN2 cores):

* All matmuls in bf16: PE streams 1 col/cycle @2.4GHz vs ~2 for f32r.
  Inputs cast host-side (ml_dtypes); PSUM accumulation stays f32.
* Softmax denominator via 64 REPLICATED ones-columns in the AV lhsT
  (v_aug = [V_h | ones64] per head): the AV matmul output [128, 512] has
  rows 0:64 = O_h^T and rows 64:128 = den_h already broadcast across 64
  partitions, aligned with the O^T rows it must normalize. Zero extra PE
  streaming time (matmul cost is N-driven), no partition broadcast, no
  DRAM bounce, no gpsimd (vs baseline's 187us tail).
* 1/den = exp(-ln den) on ACT. NOT nc.vector.reciprocal: DVE's iterative
  reciprocal measured ~3.5 cycles/element (142us/core for all pairs) and
  blocks DVE's in-order FIFO. NOT AluOpType.divide: no DVE divide on
  TRN2 (walrus NCC_IXCG864). NOT ACT Reciprocal: blocked in bass and
  can't share a table set with Exp (777+1016 > 1536 bucket budget).
* _pin_act_table_set: insert_act_table_loads otherwise assigns Exp and
  Ln to DIFFERENT table sets and thrashes 43 ACT_TABLE_LOADs (~2.7us
  each, ~116us/iteration) -- masking the functions out of all sets
  except natural_log_exp_and_others yields exactly one load.
* Emission is software-pipelined per attention jt-slot of band s:
  [AV-pair(g-1), one filler chunk, S-pair(g)] where fillers are QKV
  projection groups of band s+1 and out-projection groups of band s-1.
  The exp-gated S sits LAST so independent PE work runs while ACT
  catches up. Pair-0 slots take TWO fillers (second on the pso banks,
  free until av(0) claims them at slot 4): at band start ACT is
  backlogged with the previous band's ln/exp tail while PE has no AV
  work, so front-loading the projections there bought 8us/core. Both
  slot exps fuse into ONE N=1024 ACT op over a 2-bank [128, 2, 512]
  PSUM tile (saves ACT's 352-cycle per-op overhead).
* PSUM: psproj 1 (proj accumulators; fillers are spaced, so the single
  bank never stalls) + pss 2x2-bank (S, so S(jt+1) issues while exp(jt)
  runs) + pso 3x1-bank (AV pairs) = 8 banks exactly.

Per-band engine budgets (bf16): PE ~24us (112 matmuls; S pairs run
concurrently in separate row-groups via tile_position), ACT ~29us (16
merged exps + 12 recip ops), DVE ~17us (PSUM evictions, normalize muls,
bias adds). Measured ~40us/band steady state; residual is the
S->exp->S semaphore cadence in the attention chain.
"""

import contextlib
import sys

if '/opt/trn_rl_repo' not in sys.path:
    sys.path.insert(0, '/opt/trn_rl_repo')

import numpy as np


@contextlib.contextmanager
def _pin_act_table_set(names=("Exp", "Ln"), keep="natural_log_exp_and_others"):
    """Keep Exp+Ln servable only from the one table set that holds both, so
    insert_act_table_loads emits a single load instead of thrashing between
    exp_and_others and natural_log (~2.7us per switch, 43 switches/iter).
    Set order (= act_func_set_id indices) is preserved."""
    import concourse.bacc as bacc_mod
    import concourse.mybir as mybir
    fns = {getattr(mybir.ActivationFunctionType, n) for n in names}
    orig = bacc_mod.get_activation_tables

    def patched(arch):
        tables = dict(orig(arch))
        return {
            name: (fset if name == keep else fset - fns)
            for name, fset in tables.items()
        }

    bacc_mod.get_activation_tables = patched
    try:
        yield
    finally:
        bacc_mod.get_activation_tables = orig

P = 128
MM_DTYPE = "bf16"
NSEQ = 512
D = 512
H = 8
DH = 64
NBANDS = 7
NCORES = 8
SCALE = D ** -0.5


def build_kernel(nbands=NBANDS, repeat=1, mm_dtype=MM_DTYPE, interleave=True,
                 ablate="", tail="act"):
    import concourse.mybir as mybir
    import concourse.tile as tile
    from concourse import bacc

    f32 = mybir.dt.float32
    if mm_dtype == "bf16":
        mdt = mybir.dt.bfloat16
    elif mm_dtype == "f32r":
        mdt = mybir.dt.float32r
    else:
        mdt = mybir.dt.float32
    Exp = mybir.ActivationFunctionType.Exp
    Ln = mybir.ActivationFunctionType.Ln

    nc = bacc.Bacc("TRN2", target_bir_lowering=False, debug=False,
                   num_devices=NCORES)

    xT = nc.dram_tensor("xT", [nbands, D, NSEQ], mdt, kind="ExternalInput").ap()
    wqkvT = nc.dram_tensor("wqkvT", [D, 3 * D], mdt, kind="ExternalInput").ap()
    woutT = nc.dram_tensor("woutT", [D, D], mdt, kind="ExternalInput").ap()
    biasb = nc.dram_tensor("biasb", [P, D], f32, kind="ExternalInput").ap()
    out = nc.dram_tensor("out", [nbands, NSEQ, D], f32, kind="ExternalOutput").ap()

    with tile.TileContext(nc) as tc:
        with (
            tc.tile_pool(name="weights", bufs=1) as wpool,
            tc.tile_pool(name="x", bufs=3) as xpool,
            tc.tile_pool(name="qk", bufs=2) as qkpool,
            tc.tile_pool(name="ot", bufs=2) as otpool,
            tc.tile_pool(name="es", bufs=8) as spool,
            tc.tile_pool(name="ob", bufs=3) as outpool,
            tc.tile_pool(name="psproj", bufs=1, space="PSUM") as psproj,
            tc.tile_pool(name="pss", bufs=2, space="PSUM") as pss,
            tc.tile_pool(name="pso", bufs=3, space="PSUM") as pso,
            tc.tile_pool(name="rec", bufs=2) as recpool,
        ):
            wq_sb = wpool.tile([P, 4, 3 * D], mdt, name="wq_sb")
            wo_sb = wpool.tile([P, 4, D], mdt, name="wo_sb")
            bias_sb = wpool.tile([P, D], f32, name="bias_sb")
            # v_aug: per head 64 V columns + 64 ones columns, so the AV
            # matmul lands O^T on partitions 0:64 and the softmax
            # denominator REPLICATED on partitions 64:128. Two persistent
            # buffers alternated by band parity; ones half memset once.
            vas = [wpool.tile([P, 4, H, 2 * DH], mdt, name=f"va{i}")
                   for i in range(2)]
            for va in vas:
                ones = va[:, :, :, DH:2 * DH]
                if mm_dtype == "f32r":
                    ones = ones.bitcast(f32)
                nc.vector.memset(ones, 1.0)

            wq_r = wqkvT.rearrange("(ko ki) e -> ki ko e", ki=P)
            for kt in range(4):
                nc.sync.dma_start(wq_sb[:, kt, :], wq_r[:, kt, :])
            nc.sync.dma_start(wo_sb[:], woutT.rearrange("(ko ki) e -> ki ko e", ki=P))
            nc.sync.dma_start(bias_sb[:], biasb[:])

            def load_x(s):
                xt = xpool.tile([P, 4, NSEQ], mdt, tag="xt", name="xt")
                nc.sync.dma_start(
                    xt[:], xT[s].rearrange("(ko ki) n -> ki ko n", ki=P))
                return xt

            def qk_group(xt, qk_sb, et, boundary=False):
                pool, tag = (pso, "pso") if boundary else (psproj, "psproj")
                ps = pool.tile([P, NSEQ], f32, tag=tag, name="ps")
                for kt in range(4):
                    nc.tensor.matmul(
                        ps[:], wq_sb[:, kt, et * P:(et + 1) * P], xt[:, kt, :],
                        start=(kt == 0), stop=(kt == 3))
                nc.vector.tensor_copy(qk_sb[:, et, :], ps[:])

            def v_group(xt, va, nt, boundary=False):
                pool, tag = (pso, "pso") if boundary else (psproj, "psproj")
                ps = pool.tile([P, NSEQ], f32, tag=tag, name="ps")
                for kt in range(4):
                    nc.tensor.matmul(
                        ps[:], xt[:, kt, nt * P:(nt + 1) * P],
                        wq_sb[:, kt, 2 * D:3 * D],
                        start=(kt == 0), stop=(kt == 3))
                nc.vector.tensor_copy(
                    va[:, nt, :, 0:DH],
                    ps[:].rearrange("p (h dh) -> p h dh", h=H))

            def out_group(s, ot_sb, nt):
                ps = psproj.tile([P, NSEQ], f32, tag="psproj", name="ps")
                for kt in range(4):
                    nc.tensor.matmul(
                        ps[:], ot_sb[:, kt, nt * P:(nt + 1) * P],
                        wo_sb[:, kt, :],
                        start=(kt == 0), stop=(kt == 3))
                ob = outpool.tile([P, D], f32, tag="ob", name="ob")
                nc.vector.tensor_add(ob[:], ps[:], bias_sb[:])
                nc.sync.dma_start(
                    out[s].rearrange("(no ni) e -> ni no e", ni=P)[:, nt, :],
                    ob[:])

            def qk_chunks(s, xt):
                # q0,k0 first so the next band's pair-0 S can start early.
                for et in (0, 4, 1, 5, 2, 6, 3, 7):
                    yield lambda b=False, et=et: qk_group(xt, qk_sb_of[s],
                                                          et, boundary=b)

            def v_chunks(xt, va):
                for nt in range(4):
                    yield lambda b=False, nt=nt: v_group(xt, va, nt,
                                                         boundary=b)

            qk_sb_of = {}

            def emit_attention(s, qk_sb, va, filler):
                """S/exp for pair g interleaved with AV for pair g-1 and one
                filler chunk per jt slot; divides (the whole softmax
                normalize) inline on DVE."""
                es_store = {}
                ps_store = {}
                mul_queue = []

                def flush_muls():
                    while mul_queue:
                        g, ps_o, rec = mul_queue.pop(0)
                        nc.vector.tensor_mul(ot_sb[0:DH, g, :],
                                             ps_o[0:DH, 0, :], rec[:, 0, :])
                        nc.vector.tensor_mul(ot_sb[DH:P, g, :],
                                             ps_o[0:DH, 1, :], rec[:, 1, :])

                def s_pair(g, jt):
                    # one 2-bank PSUM tile for both halves -> ONE merged
                    # N=1024 exp (saves the 352-cycle ACT overhead per op)
                    ps_s = pss.tile([P, 2, NSEQ], f32, tag="pss", name="ps_s")
                    nc.tensor.matmul(
                        ps_s[:, 0, :], qk_sb[0:DH, 4 + g, jt * P:(jt + 1) * P],
                        qk_sb[0:DH, g, :], start=True, stop=True)
                    nc.tensor.matmul(
                        ps_s[:, 1, :], qk_sb[DH:P, 4 + g, jt * P:(jt + 1) * P],
                        qk_sb[DH:P, g, :], start=True, stop=True,
                        tile_position=(DH, 0))
                    es = spool.tile([P, 2, NSEQ], mdt, tag="es", name="es")
                    nc.scalar.activation(es[:], ps_s[:], Exp, scale=SCALE)
                    es_store.setdefault(g, []).append(es)

                def av_pair(g, jt):
                    if jt == 0:
                        ps_store[g] = (
                            pso.tile([P, NSEQ], f32, tag="pso", name="ps_o0"),
                            pso.tile([P, NSEQ], f32, tag="pso", name="ps_o1"))
                    ps_o0, ps_o1 = ps_store[g]
                    es = es_store[g][jt]
                    nc.tensor.matmul(
                        ps_o0[:], va[:, jt, 2 * g, :], es[:, 0, :],
                        start=(jt == 0), stop=(jt == 3))
                    nc.tensor.matmul(
                        ps_o1[:], va[:, jt, 2 * g + 1, :], es[:, 1, :],
                        start=(jt == 0), stop=(jt == 3))

                def divides(g):
                    # softmax normalize: rows 64:128 of ps_o hold the
                    # denominator replicated across 64 partitions, so this
                    # is a plain elementwise recip+mul -- no broadcasts.
                    # 1/den = exp(-ln den) on ACT (Ln+Exp share one table
                    # set -- pinned below so the load pass can't thrash);
                    # DVE's iterative reciprocal measured ~3.5 cyc/elem.
                    del es_store[g]
                    ps_o0, ps_o1 = ps_store.pop(g)
                    if ablate == "no_div":
                        nc.vector.tensor_copy(ot_sb[0:DH, g, :], ps_o0[0:DH, :])
                        nc.vector.tensor_copy(ot_sb[DH:P, g, :], ps_o1[0:DH, :])
                        return
                    rec = recpool.tile([DH, 2, NSEQ], f32, tag="rec",
                                       name="rec")
                    lg = recpool.tile([DH, 2, NSEQ], f32, tag="lg", name="lg")
                    nc.scalar.activation(lg[:, 0, :], ps_o0[DH:P, :], Ln)
                    nc.scalar.activation(lg[:, 1, :], ps_o1[DH:P, :], Ln)
                    nc.scalar.activation(rec[:], lg[:], Exp, scale=-1.0)
                    nc.vector.tensor_mul(ot_sb[0:DH, g, :], ps_o0[0:DH, :],
                                         rec[:, 0, :])
                    nc.vector.tensor_mul(ot_sb[DH:P, g, :], ps_o1[0:DH, :],
                                         rec[:, 1, :])

                ot_sb = otpool.tile([P, 4, NSEQ], mdt, tag="ot", name="ot_sb")
                if ablate == "no_attn":
                    nc.vector.tensor_copy(ot_sb[:], qk_sb[:, 0:4, :])
                    for f in filler:
                        f()
                    return ot_sb

                for g in range(5):
                    for jt in range(4):
                        if g >= 1:
                            av_pair(g - 1, jt)
                        if g < 4:
                            s_pair(g, jt)
                        f = next(filler, None)
                        if f is not None:
                            f()
                        if g == 0:
                            f2 = next(filler, None)
                            if f2 is not None:
                                f2(True)
                    if g >= 1:
                        divides(g - 1)
                flush_muls()
                for f in filler:
                    f()
                return ot_sb

            rep_ctx = (tc.For_i(0, repeat, 1,
                                hint_engines=(mybir.EngineType.PE,
                                              mybir.EngineType.Activation,
                                              mybir.EngineType.DVE))
                       if repeat > 1 else contextlib.nullcontext())
            with rep_ctx:
                xts = {0: load_x(0)}
                if nbands > 1:
                    xts[1] = load_x(1)
                # prologue: band 0's QKV emitted straight
                qk_sb_of[0] = qkpool.tile([P, 8, NSEQ], mdt, tag="qk",
                                          name="qk_sb")
                for f in qk_chunks(0, xts[0]):
                    f()
                for f in v_chunks(xts[0], vas[0]):
                    f()
                ot_prev = None
                for s in range(nbands):
                    if s + 2 < nbands:
                        xts[s + 2] = load_x(s + 2)
                    filler = []
                    if interleave:
                        if s + 1 < nbands:
                            qk_sb_of[s + 1] = qkpool.tile(
                                [P, 8, NSEQ], mdt, tag="qk", name="qk_sb")
                            filler.append(qk_chunks(s + 1, xts[s + 1]))
                        if ot_prev is not None:
                            op, os_ = ot_prev
                            filler.append(
                                lambda b=False, nt=nt, op=op, os_=os_:
                                    out_group(os_, op, nt)
                                for nt in range(4))
                        if s + 1 < nbands:
                            filler.append(v_chunks(xts[s + 1],
                                                   vas[(s + 1) % 2]))
                    fill_iter = (f for fl in filler for f in fl)
                    ot = emit_attention(s, qk_sb_of.pop(s), vas[s % 2],
                                        fill_iter)
                    if not interleave:
                        if s + 1 < nbands:
                            qk_sb_of[s + 1] = qkpool.tile(
                                [P, 8, NSEQ], mdt, tag="qk", name="qk_sb")
                            for f in qk_chunks(s + 1, xts[s + 1]):
                                f()
                            for f in v_chunks(xts[s + 1], vas[(s + 1) % 2]):
                                f()
                        if ot_prev is not None:
                            op, os_ = ot_prev
                            for nt in range(4):
                                out_group(os_, op, nt)
                    ot_prev = (ot, s)
                    xts.pop(s, None)
                # epilogue: last band's out-projection
                op, os_ = ot_prev
                for nt in range(4):
                    out_group(os_, op, nt)

    with _pin_act_table_set():
        nc.compile()
    return nc


_cached = None


def _get_nc():
    global _cached
    if _cached is None:
        _cached = build_kernel()
    return _cached


def make_in_maps(x, x_delta, x_theta, x_alpha, x_beta, x_gamma, x_upper,
                 Wqkv, Wout, bout, mm_dtype=MM_DTYPE):
    if mm_dtype == "bf16":
        import ml_dtypes
        cast_dt = ml_dtypes.bfloat16
    else:
        cast_dt = np.float32
    xs = np.stack([np.asarray(a, dtype=np.float32) for a in
                   (x, x_delta, x_theta, x_alpha, x_beta, x_gamma, x_upper)],
                  axis=0)  # [7, b, n, d]
    xsT = np.ascontiguousarray(xs.transpose(1, 0, 3, 2).astype(cast_dt))
    wqkvT = np.ascontiguousarray(np.asarray(Wqkv, np.float32).T.astype(cast_dt))
    woutT = np.ascontiguousarray(np.asarray(Wout, np.float32).T.astype(cast_dt))
    biasb = np.ascontiguousarray(
        np.broadcast_to(np.asarray(bout, np.float32)[None, :], (P, D)))
    return [
        {"xT": xsT[c], "wqkvT": wqkvT, "woutT": woutT, "biasb": biasb}
        for c in range(NCORES)
    ]


def kernel(x, x_delta, x_theta, x_alpha, x_beta, x_gamma, x_upper,
           Wqkv, Wout, bout):
    from concourse.bass_utils import run_bass_kernel_spmd

    nc = _get_nc()
    in_maps = make_in_maps(x, x_delta, x_theta, x_alpha, x_beta, x_gamma,
                           x_upper, Wqkv, Wout, bout)
    res = run_bass_kernel_spmd(nc, in_maps, core_ids=list(range(NCORES)))
    full = np.empty((NBANDS, NCORES, NSEQ, D), dtype=np.float32)
    for c in range(NCORES):
        full[:, c] = res.results[c]["out"]
    return tuple(full[i] for i in range(NBANDS))
